# revision 1
# baseline (speedup 1.0000x reference)
"""Trainium2 Bass kernel for GNN message passing (8 NeuronCores, SPMD).

    out = segment_sum(x[src] @ W, tgt, N) + x @ W_self

Key algebraic identity: segment_sum(x[src] @ W, tgt) = segment_sum(x[src], tgt) @ W,
so the per-edge matmul hoists out of the reduction (21 GFLOP -> 6.6 GFLOP).

Sharding: target nodes are split into 8 contiguous ranges of 12500 (one per
core); edges are bucketed to the core owning their target. x is replicated in
every core's HBM so each core gathers arbitrary source rows locally (no
cross-core halo exchange needed under the full-I/O contract).

Per core, working transposed throughout (out.T = W.T @ hT + W_self.T @ xT):
  - targets are processed in 98 windows of 128 nodes
  - per 128-edge tile: G[e,f] = x[src_e] via indirect DMA gather,
    S[e,n] = onehot(tgt_local) built by a DVE is_equal against an iota,
    hT (PSUM) += matmul(lhsT=G, rhs=S)   # = sum_e G[e,f] S[e,n]
  - outT (PSUM) = matmul(lhsT=W, rhs=hT) + matmul(lhsT=W_self, rhs=xT_window)
The host transposes per-core [128, 12544] outputs back and concatenates.
"""

import numpy as np

P = 128
D = 128
N_NODES = 100000
N_CORES = 8
N_LOC = N_NODES // N_CORES          # 12500
N_WIN = (N_LOC + P - 1) // P        # 98
N_PAD = N_WIN * P                   # 12544

# dma_gather uses int16 row indices, so x is addressed through 4 overlapping
# 32768-row chunks; every source row is reachable from >=1 chunk and rows in
# overlap regions can go to either side, which lets the host balance the four
# per-window runs under the per-chunk tile cap.
N_CHUNK = 4
CHUNK_SPAN = 32768
CHUNK_BASE = [0, 22411, 44822, N_NODES - CHUNK_SPAN]

_program_cache: dict = {}


def _build_program(
    t_win: int,
    reps: int = 1,
    n_queues: int = 2,
    act_copy: bool = True,
    w_group: int = 1,
    split16: bool = False,
):
    import concourse.bass as bass
    import concourse.mybir as mybir
    import concourse.tile as tile
    from concourse.bacc import Bacc

    f32 = mybir.dt.float32
    t_tot = N_WIN * t_win

    # consts packed as one tensor/DMA so consumers wait on a single semaphore:
    # [idx16 (int16 bits) | tl | iota | W | W_self] along the free dim
    t_c = t_win // N_CHUNK
    idx_cols16 = N_WIN * N_CHUNK * t_c * 8   # int16 columns
    idx_cols = idx_cols16 // 2               # as float32 columns
    k_const = idx_cols + t_tot + 3 * P

    # Bacc (not raw Bass): its finalize() legalizes sync waits — TRN2 allows
    # at most one semaphore wait per instruction and walrus rejects more.
    nc = Bacc(num_swdge_queues=n_queues)
    bf16 = mybir.dt.bfloat16
    if split16:
        # x pre-split on host into [bf16(x) | bf16(x - bf16(x))] per row: the
        # aggregation runs as two bf16 matmuls (G_hi + G_lo vs exact-bf16 S),
        # streaming at 1 cy/row each vs fp32's 4 cy/row — 2x PE with ~17-bit
        # effective mantissa. Same gather descriptor count and bytes (512B/row).
        xs_d = nc.declare_dram_parameter(
            "xs", [N_NODES, 2 * D], bf16, isOutput=False
        )
    else:
        x_d = nc.declare_dram_parameter("x", [N_NODES, D], f32, isOutput=False)
    xT_d = nc.declare_dram_parameter("xT", [D, N_PAD], f32, isOutput=False)
    consts_d = nc.declare_dram_parameter(
        "consts", [P, k_const], mybir.dt.int32, isOutput=False
    )
    outT_d = nc.declare_dram_parameter("outT", [D, N_PAD], f32, isOutput=True)

    with tile.TileContext(nc) as tc:
        with (
            tc.tile_pool(name="const", bufs=1) as cpool,
            tc.tile_pool(name="gath", bufs=16) as gpool,
            tc.tile_pool(name="spool", bufs=3) as spool,
            tc.tile_pool(name="wtile", bufs=3) as wpool,
            tc.tile_pool(name="psum", bufs=2, space="PSUM") as psum,
            tc.tile_pool(name="opsum", bufs=2, space="PSUM") as opsum,
            tc.tile_pool(name="scratch", bufs=1, space="PSUM") as scratch_pool,
        ):
            scratch_ps = scratch_pool.tile([1, 1], f32)
            const_sb = cpool.tile([P, k_const], mybir.dt.int32)
            nc.sync.dma_start(const_sb[:], consts_d[:])
            idx16_sb = const_sb[:, 0:idx_cols].bitcast(mybir.dt.int16)
            tl_sb = const_sb[:, idx_cols : idx_cols + t_tot].bitcast(f32)
            iota_sb = const_sb[:, idx_cols + t_tot : idx_cols + t_tot + P].bitcast(f32)
            w_sb = const_sb[:, idx_cols + t_tot + P : idx_cols + t_tot + 2 * P].bitcast(
                f32
            )
            ws_sb = const_sb[
                :, idx_cols + t_tot + 2 * P : idx_cols + t_tot + 3 * P
            ].bitcast(f32)

            s_dt = bf16 if split16 else f32
            for w in [w for _ in range(reps) for w in range(N_WIN)]:
                hT_ps = psum.tile([D, P], f32)
                S_big = spool.tile([P, t_win, P], s_dt)
                nc.vector.tensor_tensor(
                    out=S_big[:],
                    in0=tl_sb[:, w * t_win : (w + 1) * t_win, None].to_broadcast(
                        [P, t_win, P]
                    ),
                    in1=iota_sb[:, None, :].to_broadcast([P, t_win, P]),
                    op=mybir.AluOpType.is_equal,
                )
                # fp32 matmuls are single fused instructions that can carry only
                # ONE sync wait; the first real matmul below depends on both
                # S_big (DVE) and G_big (DMA). This throwaway 1x1 matmul makes
                # the PE queue observe the DVE tick first so each real matmul
                # needs a single wait.
                nc.tensor.matmul(
                    scratch_ps[:],
                    lhsT=S_big[:, 0, 0:1],
                    rhs=S_big[:, 0, 0:1],
                    start=True,
                    stop=True,
                )
                # gather via dma_gather (int16 idx against a 32768-row chunk of
                # x): G_big slot (p, t) = row idx[t*128+p] of the chunk. Much
                # cheaper on the GPSIMD Q7 descriptor generator than per-tile
                # indirect_dma_start. (A single batched indirect DMA with a
                # [128, t_win] offset AP computes garbage on real HW.)
                gd = 2 * D if split16 else D
                G_big = gpool.tile([P, t_win, gd], s_dt)
                for c in range(N_CHUNK):
                    g = w * N_CHUNK + c
                    nc.gpsimd.dma_gather(
                        G_big[:, c * t_c : (c + 1) * t_c, :],
                        (xs_d if split16 else x_d)[
                            CHUNK_BASE[c] : CHUNK_BASE[c] + CHUNK_SPAN, :
                        ],
                        idx16_sb[:, g * t_c * 8 : (g + 1) * t_c * 8],
                        t_c * P,
                        t_c * P,
                        gd,
                        queue_num=c % n_queues,
                    )
                for t in range(t_win):
                    if split16:
                        nc.tensor.matmul(
                            hT_ps[:],
                            lhsT=G_big[:, t, 0:D],
                            rhs=S_big[:, t, :],
                            start=(t == 0),
                            stop=False,
                        )
                        nc.tensor.matmul(
                            hT_ps[:],
                            lhsT=G_big[:, t, D : 2 * D],
                            rhs=S_big[:, t, :],
                            start=False,
                            stop=(t == t_win - 1),
                        )
                    else:
                        nc.tensor.matmul(
                            hT_ps[:],
                            lhsT=G_big[:, t, :],
                            rhs=S_big[:, t, :],
                            start=(t == 0),
                            stop=(t == t_win - 1),
                        )
                if w_group == 1:
                    hT_sb = wpool.tile([D, P], f32, tag="hT")
                    nc.vector.tensor_copy(hT_sb[:], hT_ps[:])
                    xT_sb = wpool.tile([D, P], f32, tag="xT")
                    nc.sync.dma_start(xT_sb[:], xT_d[:, w * P : (w + 1) * P])
                    outT_ps = opsum.tile([D, P], f32)
                    nc.tensor.matmul(
                        outT_ps[:], lhsT=w_sb, rhs=hT_sb[:], start=True, stop=False
                    )
                    nc.tensor.matmul(
                        outT_ps[:], lhsT=ws_sb, rhs=xT_sb[:], start=False, stop=True
                    )
                    o_sb = wpool.tile([D, P], f32, tag="o")
                    if act_copy:
                        # ACT is otherwise idle; taking the outT copy off DVE
                        # (which builds every S one-hot) relieves the
                        # 2nd-busiest engine despite slower per-op copies.
                        nc.scalar.copy(o_sb[:], outT_ps[:])
                    else:
                        nc.vector.tensor_copy(o_sb[:], outT_ps[:])
                    nc.sync.dma_start(outT_d[:, w * P : (w + 1) * P], o_sb[:])
                    continue
                # grouped W-apply: stage hT of w_group windows side by side,
                # then stream both weight matmuls at N = w_group*128 to
                # amortize the fp32 weight loads (no FWL for fp32)
                gi = w % w_group
                if gi == 0:
                    n_in_grp = min(w_group, N_WIN - w)
                    hT_sb = wpool.tile([D, w_group * P], f32, tag="hT")
                nc.vector.tensor_copy(
                    hT_sb[:, gi * P : (gi + 1) * P], hT_ps[:]
                )
                if gi == n_in_grp - 1:
                    w0 = w - gi
                    span = n_in_grp * P
                    xT_sb = wpool.tile([D, w_group * P], f32, tag="xT")
                    nc.sync.dma_start(
                        xT_sb[:, :span], xT_d[:, w0 * P : w0 * P + span]
                    )
                    outT_ps = opsum.tile([D, w_group * P], f32)
                    nc.tensor.matmul(
                        outT_ps[:, :span],
                        lhsT=w_sb,
                        rhs=hT_sb[:, :span],
                        start=True,
                        stop=False,
                    )
                    nc.tensor.matmul(
                        outT_ps[:, :span],
                        lhsT=ws_sb,
                        rhs=xT_sb[:, :span],
                        start=False,
                        stop=True,
                    )
                    o_sb = wpool.tile([D, w_group * P], f32, tag="o")
                    if act_copy:
                        nc.scalar.copy(o_sb[:, :span], outT_ps[:, :span])
                    else:
                        nc.vector.tensor_copy(o_sb[:, :span], outT_ps[:, :span])
                    nc.sync.dma_start(
                        outT_d[:, w0 * P : w0 * P + span], o_sb[:, :span]
                    )

    nc.finalize()
    return nc


def _prep_inputs(x, edge_index, W, W_self):
    """Host-side sharding: bucket+sort edges by target core/window, pad to a
    uniform tile count, build per-core input maps."""
    x = np.ascontiguousarray(np.asarray(x, dtype=np.float32))
    W = np.ascontiguousarray(np.asarray(W, dtype=np.float32))
    W_self = np.ascontiguousarray(np.asarray(W_self, dtype=np.float32))
    ei = np.asarray(edge_index)
    src = ei[0].astype(np.int64)
    tgt = ei[1].astype(np.int64)
    E = src.shape[0]

    order = np.argsort(tgt, kind="stable")
    src_s = src[order].astype(np.int64)
    tgt_s = tgt[order]
    core = tgt_s // N_LOC
    wloc = (tgt_s - core * N_LOC) // P
    gw = (core * N_WIN + wloc).astype(np.int64)
    counts = np.bincount(gw, minlength=N_CORES * N_WIN)
    t_win_data = max(1, int(np.ceil(counts.max() / P)))
    t_c = max(2, (t_win_data + N_CHUNK - 1) // N_CHUNK)

    # chunk feasibility per edge: lo = highest chunk with base <= s,
    # hi = lowest chunk with s < base + CHUNK_SPAN (consecutive range)
    bases = np.asarray(CHUNK_BASE, np.int64)
    lo = np.searchsorted(bases, src_s, side="right") - 1
    hi = np.searchsorted(bases + CHUNK_SPAN, src_s, side="right")
    starts = np.concatenate([[0], np.cumsum(counts)])
    tl_val = (tgt_s - (core * N_LOC + wloc * P)).astype(np.float32)

    while True:
        cap = t_c * P
        t_win = N_CHUNK * t_c
        t_tot = N_WIN * t_win
        idx16 = np.zeros((N_CORES, N_WIN * N_CHUNK * cap // 16, 16), np.int16)
        tl_flat = np.full(N_CORES * t_tot * P, -1.0, np.float32)
        ok = True
        for g in range(N_CORES * N_WIN):
            a, b = starts[g], starts[g + 1]
            if b - a > N_CHUNK * cap:
                ok = False
                break
            s_g, hi_g, lo_g, tl_g = src_s[a:b], hi[a:b], lo[a:b], tl_val[a:b]
            taken = np.zeros(b - a, bool)
            c_core, w = divmod(g, N_WIN)
            for c in range(N_CHUNK):
                cand = (~taken) & (hi_g <= c) & (c <= lo_g)
                must = cand & (lo_g == c)
                n_must = int(must.sum())
                if n_must > cap:
                    ok = False
                    break
                sel = must.nonzero()[0]
                flex = (cand & ~must).nonzero()[0][: cap - n_must]
                pick = np.concatenate([sel, flex])
                taken[pick] = True
                n = pick.size
                idx = (s_g[pick] - bases[c]).astype(np.int16)
                # wrapped int16 layout: slot s -> [s % 16, s // 16]
                blk = np.zeros(cap, np.int16)
                blk[:n] = idx
                row0 = (w * N_CHUNK + c) * (cap // 16)
                idx16[c_core, row0 : row0 + cap // 16] = blk.reshape(cap // 16, 16)
                # tl slots for this chunk run (pads stay -1)
                base_slot = g * (t_win * P) + c * cap
                tl_flat[base_slot : base_slot + n] = tl_g[pick]
            if not ok or not taken.all():
                ok = ok and bool(taken.all())
                if not ok:
                    break
        if ok:
            break
        t_c += 1

    tl_dev = tl_flat.reshape(N_CORES, t_tot, P).transpose(0, 2, 1)
    iota = np.tile(np.arange(P, dtype=np.float32), (P, 1))
    in_maps = []
    for c in range(N_CORES):
        # idx16[c]: [n16, 16] with slot s of block g at [g*cap/16 + s%16 ...]
        # -> SBUF layout [128 partitions, cols]: block g occupies columns
        # [g*t_c*8, (g+1)*t_c*8), partition rows 0..15
        n_blocks = N_WIN * N_CHUNK
        cols16 = t_c * 8
        a = idx16[c].reshape(n_blocks, cap // 16, 16)  # [g, col, row]
        # [16, cols] block replicated across all 8 GPSIMD Q7 cores' stripes
        sb = np.tile(a.transpose(2, 0, 1).reshape(16, n_blocks * cols16), (8, 1))
        if c == 0:
            import ml_dtypes

            x_hi = x.astype(ml_dtypes.bfloat16)
            x_lo = (x - x_hi.astype(np.float32)).astype(ml_dtypes.bfloat16)
            xs = np.concatenate([x_hi, x_lo], axis=1)
        xT_c = np.zeros((D, N_PAD), np.float32)
        xT_c[:, :N_LOC] = x[c * N_LOC : (c + 1) * N_LOC].T
        consts = np.concatenate(
            [
                sb.view(np.int32),
                tl_dev[c].view(np.int32),
                iota.view(np.int32),
                W.view(np.int32),
                W_self.view(np.int32),
            ],
            axis=1,
        )
        in_maps.append({"x": x, "xs": xs, "xT": xT_c, "consts": consts})
    return in_maps, t_win


def run(x, edge_index, W, W_self, trace=False, **trace_kwargs):
    """Returns (output [100000,128] float32, BassKernelResults)."""
    from concourse import bass_utils

    in_maps, t_win = _prep_inputs(x, edge_index, W, W_self)
    nc = _program_cache.get(t_win)
    if nc is None:
        nc = _build_program(t_win)
        _program_cache[t_win] = nc
    # A NeuronCore occasionally comes up wedged from a previous session
    # (NRT_EXEC_UNIT_UNRECOVERABLE); the failed attempt itself clears it, so
    # one retry recovers.
    try:
        res = bass_utils.run_bass_kernel_spmd(
            nc, in_maps, core_ids=list(range(N_CORES)), trace=trace, **trace_kwargs
        )
    except Exception:
        res = bass_utils.run_bass_kernel_spmd(
            nc, in_maps, core_ids=list(range(N_CORES)), trace=trace, **trace_kwargs
        )
    out = np.empty((N_NODES, D), np.float32)
    for c in range(N_CORES):
        out[c * N_LOC : (c + 1) * N_LOC] = res.results[c]["outT"].T[:N_LOC]
    return out, res


def kernel(x, edge_index, W, W_self):
    out, _ = run(x, edge_index, W, W_self, trace=False)
    return out



# revision 7
# speedup vs baseline: 2.0685x; 2.0685x over previous
"""Trainium2 Bass kernel for GNN message passing (8 NeuronCores, SPMD).

    out = segment_sum(x[src] @ W, tgt, N) + x @ W_self

Key algebraic identity: segment_sum(x[src] @ W, tgt) = segment_sum(x[src], tgt) @ W,
so the per-edge matmul hoists out of the reduction (21 GFLOP -> 6.6 GFLOP).

Sharding: target nodes are split into 8 contiguous ranges of 12500 (one per
core); edges are bucketed to the core owning their target. Windows of 128
targets are processed in groups of 14; for each (core, group) the host
collects the group's distinct source rows into a contiguous block of a
per-core bf16 source tensor xs (the "halo" for that partition of the graph),
so the whole group needs ONE dma_gather call (SWDGE has ~1us fixed cost per
call, so call count dominates descriptor count).

Per core, working transposed throughout (out.T = W.T @ hT + W_self.T @ xT),
all in bf16 (rel tolerance 2e-2; bf16 keeps ~3e-3):
  - per group: G[e,f] slab = xs[idx_e] via one indirect gather (one 256B
    descriptor per edge slot)
  - per window (t_w tiles of 128 edge slots): S[e, j, t] =
    is_equal(tl[e,t], j) built by DVE in [slot-lo, target, tile] layout --
    all operands 2-byte with packed last dim, which qualifies for the DVE
    2x_1p fast mode (the [e, t, j] layout broadcasts tl along the last dim
    and runs 1x)
  - hT (PSUM) += matmul(lhsT=G_tile, rhs=S[:, :, t])  # = sum_e G[e,f] S[e,j]
  - per 4 windows: outT (PSUM) = matmul(lhsT=W, rhs=hT) +
    matmul(lhsT=W_self, rhs=xT_window); ACT copies hT out of PSUM (cast to
    bf16), DVE copies outT (cast to bf16)
The host transposes per-core [128, 12544] bf16 outputs back and concatenates.
"""

import numpy as np

P = 128
D = 128
N_NODES = 100000
N_CORES = 8
N_LOC = N_NODES // N_CORES          # 12500
N_WIN = (N_LOC + P - 1) // P        # 98
N_PAD = N_WIN * P                   # 12544
W_GRP = 14                          # windows per gather group (98 = 7*14)
N_GRP = N_WIN // W_GRP              # 7
WAPPLY = 4                          # windows per W-apply / output DMA group

_program_cache: dict = {}


def _build_program(
    layout,
    reps: int = 1,
    n_queues: int = 2,
    gather_split: int = 8,
    dma_scratch: int = 16384,
):
    import concourse.bass as bass
    import concourse.mybir as mybir
    import concourse.tile as tile
    from concourse.bacc import Bacc

    f32 = mybir.dt.float32
    bf16 = mybir.dt.bfloat16

    t = layout["t"]                  # tiles per window, len 98
    LEN_G = layout["LEN_G"]          # rows per group source block
    T_MAX = layout["T_MAX"]
    t_tot = layout["t_tot"]          # sum(t), padded even
    grp_tiles = [sum(t[g * W_GRP : (g + 1) * W_GRP]) for g in range(N_GRP)]
    TILES_MAX = max(grp_tiles)
    idx_cols32 = layout["idx_cols32"]
    k_const = layout["k_const"]

    nc = Bacc(num_swdge_queues=n_queues, dynamic_dma_scratch_size=dma_scratch)
    xs_d = nc.declare_dram_parameter("xs", [N_GRP * LEN_G, D], bf16, isOutput=False)
    xT_d = nc.declare_dram_parameter("xT", [D, N_PAD], bf16, isOutput=False)
    iotaB_d = nc.declare_dram_parameter("iotaB", [P, P, T_MAX], bf16, isOutput=False)
    consts_d = nc.declare_dram_parameter(
        "consts", [P, k_const], mybir.dt.int32, isOutput=False
    )
    outT_d = nc.declare_dram_parameter("outT", [D, N_PAD], bf16, isOutput=True)

    with tile.TileContext(nc) as tc:
        with (
            tc.tile_pool(name="const", bufs=1) as cpool,
            tc.tile_pool(name="gath", bufs=2) as gpool,
            tc.tile_pool(name="spool", bufs=4) as spool,
            tc.tile_pool(name="wtile", bufs=3) as wpool,
            tc.tile_pool(name="psum", bufs=4, space="PSUM") as psum,
            tc.tile_pool(name="opsum", bufs=2, space="PSUM") as opsum,
            tc.tile_pool(name="scratch", bufs=1, space="PSUM") as scratch_pool,
        ):
            scratch_ps = scratch_pool.tile([1, 1], f32)
            const_sb = cpool.tile([P, k_const], mybir.dt.int32)
            nc.sync.dma_start(const_sb[:], consts_d[:])
            iotaB_sb = cpool.tile([P, P, T_MAX], bf16)
            nc.sync.dma_start(iotaB_sb[:], iotaB_d[:])
            idx16_sb = const_sb[:, 0:idx_cols32].bitcast(mybir.dt.int16)
            tl_sb = const_sb[:, idx_cols32 : idx_cols32 + t_tot // 2].bitcast(bf16)
            w_sb = const_sb[
                :, idx_cols32 + t_tot // 2 : idx_cols32 + t_tot // 2 + 64
            ].bitcast(bf16)
            ws_sb = const_sb[
                :, idx_cols32 + t_tot // 2 + 64 : idx_cols32 + t_tot // 2 + 128
            ].bitcast(bf16)

            for rep in range(reps):
                toff = 0
                hT_sb = None
                for g in range(N_GRP):
                    TILES_g = grp_tiles[g]
                    G = gpool.tile([P, TILES_MAX, D], bf16)
                    ioff = sum(gt * 8 for gt in grp_tiles[:g])  # int16 cols
                    if gather_split and TILES_g > gather_split:
                        # split the group's gather into chunks of <= split tiles
                        t0 = 0
                        q = 0
                        while t0 < TILES_g:
                            tn = min(gather_split, TILES_g - t0)
                            nc.gpsimd.dma_gather(
                                G[:, t0 : t0 + tn, :],
                                xs_d[g * LEN_G : (g + 1) * LEN_G, :],
                                idx16_sb[:, ioff + t0 * 8 : ioff + (t0 + tn) * 8],
                                tn * P,
                                tn * P,
                                D,
                                queue_num=(g + q) % n_queues,
                            )
                            t0 += tn
                            q += 1
                    else:
                        nc.gpsimd.dma_gather(
                            G[:, :TILES_g, :],
                            xs_d[g * LEN_G : (g + 1) * LEN_G, :],
                            idx16_sb[:, ioff : ioff + TILES_g * 8],
                            TILES_g * P,
                            TILES_g * P,
                            D,
                            queue_num=g % n_queues,
                        )
                    goff = 0
                    for wi in range(W_GRP):
                        w = g * W_GRP + wi
                        t_w = t[w]
                        S = spool.tile([P, P, T_MAX], bf16)
                        nc.vector.tensor_tensor(
                            out=S[:, :, 0:t_w],
                            in0=tl_sb[:, None, toff : toff + t_w].to_broadcast(
                                [P, P, t_w]
                            ),
                            in1=iotaB_sb[:, :, 0:t_w],
                            op=mybir.AluOpType.is_equal,
                        )
                        hT_ps = psum.tile([D, P], f32)
                        # bf16 matmuls carry one sync wait; the first real
                        # matmul depends on both S (DVE) and G (gather DMA).
                        # The throwaway matmul makes PE observe the DVE tick
                        # first so each real matmul needs a single wait.
                        nc.tensor.matmul(
                            scratch_ps[:],
                            lhsT=S[:, 0, 0:1],
                            rhs=S[:, 0, 0:1],
                            start=True,
                            stop=True,
                        )
                        for tt in range(t_w):
                            nc.tensor.matmul(
                                hT_ps[:],
                                lhsT=G[:, goff + tt, :],
                                rhs=S[:, :, tt],
                                start=(tt == 0),
                                stop=(tt == t_w - 1),
                            )
                        goff += t_w
                        toff += t_w
                        # grouped W-apply: stage hT of WAPPLY windows side by
                        # side (ACT copies out of PSUM, casting to bf16), then
                        # stream both weight matmuls at N = WAPPLY*128
                        gi = w % WAPPLY
                        if gi == 0:
                            n_in_grp = min(WAPPLY, N_WIN - w)
                            hT_sb = wpool.tile([D, WAPPLY * P], bf16, tag="hT")
                        nc.scalar.copy(hT_sb[:, gi * P : (gi + 1) * P], hT_ps[:])
                        if gi == n_in_grp - 1:
                            w0 = w - gi
                            span = n_in_grp * P
                            xT_sb = wpool.tile([D, WAPPLY * P], bf16, tag="xT")
                            nc.sync.dma_start(
                                xT_sb[:, :span], xT_d[:, w0 * P : w0 * P + span]
                            )
                            outT_ps = opsum.tile([D, WAPPLY * P], f32)
                            nc.tensor.matmul(
                                outT_ps[:, :span],
                                lhsT=w_sb,
                                rhs=hT_sb[:, :span],
                                start=True,
                                stop=False,
                            )
                            nc.tensor.matmul(
                                outT_ps[:, :span],
                                lhsT=ws_sb,
                                rhs=xT_sb[:, :span],
                                start=False,
                                stop=True,
                            )
                            o_sb = wpool.tile([D, WAPPLY * P], bf16, tag="o")
                            nc.vector.tensor_copy(o_sb[:, :span], outT_ps[:, :span])
                            nc.sync.dma_start(
                                outT_d[:, w0 * P : w0 * P + span], o_sb[:, :span]
                            )

    nc.finalize()
    return nc


def _prep_inputs(x, edge_index, W, W_self):
    """Host-side sharding: bucket+sort edges by target core/window, group
    windows, build per-(core, group) compacted source blocks and index/one-hot
    metadata."""
    import ml_dtypes

    bf16 = ml_dtypes.bfloat16
    x = np.ascontiguousarray(np.asarray(x, dtype=np.float32))
    x_bf = x.astype(bf16)
    W_bf = np.ascontiguousarray(np.asarray(W, dtype=np.float32)).astype(bf16)
    Ws_bf = np.ascontiguousarray(np.asarray(W_self, dtype=np.float32)).astype(bf16)
    ei = np.asarray(edge_index)
    src = ei[0].astype(np.int64)
    tgt = ei[1].astype(np.int64)

    order = np.argsort(tgt, kind="stable")
    src_s = src[order]
    tgt_s = tgt[order]
    core = tgt_s // N_LOC
    wloc = (tgt_s - core * N_LOC) // P
    gw = (core * N_WIN + wloc).astype(np.int64)
    counts = np.bincount(gw, minlength=N_CORES * N_WIN).reshape(N_CORES, N_WIN)
    starts = np.concatenate([[0], np.cumsum(counts.reshape(-1))])

    # uniform-across-cores tiles per window (program is shared SPMD)
    t = np.maximum(2, -(-counts.max(axis=0) // P)).astype(np.int64)  # [N_WIN]
    T_MAX = int(t.max())
    t_tot = int(t.sum())
    if t_tot % 2:
        t = t.copy()
        t[-1] += 1
        t_tot += 1
        T_MAX = max(T_MAX, int(t[-1]))

    # per (core, group) distinct source blocks
    grp_src: list[list[np.ndarray]] = [[] for _ in range(N_CORES)]
    grp_rank: list[list[np.ndarray]] = [[] for _ in range(N_CORES)]
    len_g = np.zeros((N_CORES, N_GRP), np.int64)
    for c in range(N_CORES):
        for g in range(N_GRP):
            a = starts[c * N_WIN + g * W_GRP]
            b = starts[c * N_WIN + (g + 1) * W_GRP]
            u, inv = np.unique(src_s[a:b], return_inverse=True)
            grp_src[c].append(u)
            grp_rank[c].append(inv)
            len_g[c, g] = len(u)
    LEN_G = int(len_g.max())
    assert LEN_G <= 32767, LEN_G

    num_idxs_g = [int(t[g * W_GRP : (g + 1) * W_GRP].sum()) * P for g in range(N_GRP)]
    idx_cols16 = sum(num_idxs_g) // 16
    idx_cols32 = idx_cols16 // 2
    k_const = idx_cols32 + t_tot // 2 + 128

    iotaB = np.broadcast_to(
        np.arange(P, dtype=np.float32).astype(bf16)[None, :, None], (P, P, T_MAX)
    ).copy()

    in_maps = []
    for c in range(N_CORES):
        xs = np.zeros((N_GRP * LEN_G, D), bf16)
        idx16 = np.zeros((16, idx_cols16), np.int16)
        tl_img = np.full((P, t_tot), -1.0, bf16)
        icol = 0
        toff = 0
        for g in range(N_GRP):
            u = grp_src[c][g]
            xs[g * LEN_G : g * LEN_G + len(u)] = x_bf[u]
            ranks = grp_rank[c][g]
            a0 = starts[c * N_WIN + g * W_GRP]
            call_idx = np.zeros(num_idxs_g[g], np.int16)
            soff = 0
            for wi in range(W_GRP):
                w = g * W_GRP + wi
                a = starts[c * N_WIN + w] - a0
                b = starts[c * N_WIN + w + 1] - a0
                n = b - a
                cap = int(t[w]) * P
                assert n <= cap, (c, w, n, cap)
                r = ranks[a:b]
                perm = np.argsort(r)
                call_idx[soff : soff + n] = r[perm].astype(np.int16)
                # target-local id within the window
                tlv = (tgt_s[a0 + a : a0 + b] % N_LOC - w * P).astype(np.float32)
                tl_img[:, toff : toff + int(t[w])] = (
                    np.pad(tlv[perm], (0, cap - n), constant_values=-1.0)
                    .reshape(int(t[w]), P)
                    .T.astype(bf16)
                )
                soff += cap
                toff += int(t[w])
            cols = num_idxs_g[g] // 16
            idx16[:, icol : icol + cols] = call_idx.reshape(cols, 16).T
            icol += cols
        idx_rep = np.tile(idx16, (8, 1))  # replicate across the 8 Q7 stripes
        xT_c = np.zeros((D, N_PAD), bf16)
        xT_c[:, :N_LOC] = x_bf[c * N_LOC : (c + 1) * N_LOC].T
        consts = np.concatenate(
            [
                idx_rep.view(np.int32),
                tl_img.view(np.int32),
                np.ascontiguousarray(W_bf).view(np.int32),
                np.ascontiguousarray(Ws_bf).view(np.int32),
            ],
            axis=1,
        )
        assert consts.shape == (P, k_const), (consts.shape, k_const)
        in_maps.append(
            {"xs": xs, "xT": xT_c, "iotaB": iotaB, "consts": consts}
        )

    layout = {
        "t": [int(v) for v in t],
        "LEN_G": LEN_G,
        "T_MAX": T_MAX,
        "t_tot": t_tot,
        "idx_cols32": idx_cols32,
        "k_const": k_const,
    }
    return in_maps, layout


def run(x, edge_index, W, W_self, trace=False, **trace_kwargs):
    """Returns (output [100000,128] float32, BassKernelResults)."""
    from concourse import bass_utils

    in_maps, layout = _prep_inputs(x, edge_index, W, W_self)
    key = tuple(layout["t"]) + (layout["LEN_G"],)
    nc = _program_cache.get(key)
    if nc is None:
        nc = _build_program(layout)
        _program_cache[key] = nc
    # A NeuronCore occasionally comes up wedged from a previous session
    # (NRT_EXEC_UNIT_UNRECOVERABLE); the failed attempt itself clears it, so
    # one retry recovers.
    try:
        res = bass_utils.run_bass_kernel_spmd(
            nc, in_maps, core_ids=list(range(N_CORES)), trace=trace, **trace_kwargs
        )
    except Exception:
        res = bass_utils.run_bass_kernel_spmd(
            nc, in_maps, core_ids=list(range(N_CORES)), trace=trace, **trace_kwargs
        )
    out = np.empty((N_NODES, D), np.float32)
    for c in range(N_CORES):
        out[c * N_LOC : (c + 1) * N_LOC] = (
            res.results[c]["outT"].astype(np.float32).T[:N_LOC]
        )
    return out, res


def kernel(x, edge_index, W, W_self):
    out, _ = run(x, edge_index, W, W_self, trace=False)
    return out
